# revision 1
# baseline (speedup 1.0000x reference)
"""Trainium2 Bass kernel for nn_CountingAbstraction (sparse_attention).

Math (per batch b):
    cn  = l2_normalize(data[b], axis=-1)
    sim = relu(cn @ cn.T)                       # [N, N]
    counter_pre = sim @ [1 | fixed_v]           # rowsum + sim@posenc, [N, 513]
    counter = softplus(counter_pre @ W_exp + b_exp)
    out = [data | counter] @ W_merge

Device formulation (flash-attention-style fusion, never materializing sim):
    Wt = fixed_v @ W_exp[1:] + 1*W_exp[0]       # [N, M], folds rowsum+Dense
    z.T[m, q] = sum_k Wt[k, m] * relu(cnT_k.T @ cnT_q)[k, q]
    counter.T = softplus(z.T + b_exp)           # per-partition bias
    out[q, :] = dataT_q.T @ W_merge[:D] + counter.T.T @ W_merge[D:]

Sharding: core c handles batch c//2, query rows half c%2 (2048 rows) against
all 4096 keys of that batch. Data-parallel, no collectives.

Matmuls run in bf16 (fp32 PSUM accumulation). data arrives host-cast to bf16
(halves the startup DMA; norms computed from bf16 are within ~1e-4 because the
sum-of-squares averages 512 positive rounding errors). softplus is computed as
relu(z+b) [DVE] + ln(1 + exp(-|z+b|)) [ACT], which is range-safe, and the
merge matmuls of chunk ch-1 are emitted between the k-loop and softplus of
chunk ch so the in-order PE stream has work while ACT runs the softplus chain.
"""

import sys

for _p in ("/opt/trn_rl_repo",):
    if _p not in sys.path:
        sys.path.insert(0, _p)

import numpy as np
import ml_dtypes

import concourse.tile as tile
import concourse.mybir as mybir
from concourse import bacc
from concourse.bass import ts, ds
from concourse.bass_utils import run_bass_kernel_spmd

F32 = mybir.dt.float32
BF16 = mybir.dt.bfloat16
AF = mybir.ActivationFunctionType
ALU = mybir.AluOpType
BF = ml_dtypes.bfloat16

B, N, D, M = 4, 4096, 512, 512
NCORES = 8
NQ = (B * N) // NCORES  # 2048 query rows per core


def _posenc(n, d):
    pos = np.arange(n, dtype=np.float32)[:, None]
    i = np.arange(d // 2, dtype=np.float32)[None, :]
    angle = pos / np.power(10000.0, 2.0 * i / d)
    pe = np.zeros((n, d), dtype=np.float32)
    pe[:, 0::2] = np.sin(angle)
    pe[:, 1::2] = np.cos(angle)
    return pe


def _chunks(total, size):
    off = 0
    while off < total:
        w = min(size, total - off)
        yield off, w
        off += w


def build_nc(nkeys=N, nq=NQ, qch=512, num_cores=NCORES):
    """Build the SPMD Bass kernel (identical on every core)."""
    assert D % 128 == 0 and M % 128 == 0 and nkeys % 512 == 0
    assert nq % qch == 0 and qch % 128 == 0 and qch <= 512
    assert nq % 512 == 0  # rawq capture is per 512-column key chunk
    DP = D // 128       # contraction subtiles over feature dim
    MJ = M // 128       # output-column subtiles
    KB = nkeys // 128   # key blocks
    NCH = nq // qch     # query chunks

    nc = bacc.Bacc("TRN2", target_bir_lowering=False, debug=False,
                   num_devices=num_cores)
    dTk = nc.dram_tensor("dTk", [D, nkeys], BF16, kind="ExternalInput").ap()
    fvT = nc.dram_tensor("fvT", [D, nkeys], BF16, kind="ExternalInput").ap()
    wexp1 = nc.dram_tensor("wexp1", [D, M], BF16, kind="ExternalInput").ap()
    w0 = nc.dram_tensor("w0", [1, M], F32, kind="ExternalInput").ap()
    wm = nc.dram_tensor("wm", [D + M, M], BF16, kind="ExternalInput").ap()
    bexp = nc.dram_tensor("bexp", [MJ, 128], F32, kind="ExternalInput").ap()
    out = nc.dram_tensor("out", [nq, M], F32, kind="ExternalOutput").ap()

    with tile.TileContext(nc) as tc:
        with (
            tc.tile_pool(name="res", bufs=1) as res,
            tc.tile_pool(name="trans", bufs=2) as trans,
            tc.tile_pool(name="work", bufs=3) as work,
            tc.tile_pool(name="psg", bufs=3, space="PSUM") as psg,
            tc.tile_pool(name="psz", bufs=MJ, space="PSUM") as psz,
            tc.tile_pool(name="pso", bufs=1, space="PSUM") as pso,
        ):
            # ---- constants / residents -------------------------------------
            wexp_sb = res.tile([128, DP, M], BF16, tag="wexp", name="wexp_sb")
            for c in range(D // 128):
                nc.sync.dma_start(wexp_sb[:, c, :], wexp1[ts(c, 128), :])
            wm_sb = res.tile([128, DP + MJ, M], BF16, tag="wm", name="wm_sb")
            bexp_sb = res.tile([128, MJ], F32, tag="bexp", name="bexp_sb")
            nc.sync.dma_start(bexp_sb[:], bexp.rearrange("c p -> p c"))
            w0_sb = res.tile([1, M], F32, tag="w0", name="w0_sb")
            nc.sync.dma_start(w0_sb[:], w0[:])
            w0b = res.tile([128, M], F32, tag="w0b", name="w0b")
            nc.gpsimd.partition_broadcast(w0b[:], w0_sb[:])
            ones_col = res.tile([128, 1], BF16, tag="ones", name="ones_col")
            nc.vector.memset(ones_col[:], 1.0)

            wt = res.tile([128, KB, M], BF16, tag="wt", name="wt")
            cnk = res.tile([128, DP, nkeys], BF16, tag="cnk", name="cnk")
            rawq = res.tile([128, DP, nq], BF16, tag="rawq", name="rawq")
            n_row_k = res.tile([1, nkeys], F32, tag="nrk", name="n_row_k")

            # ---- phase A: Wt = fvT.T @ wexp1 + 1*w0 ------------------------
            # The first two groups are emitted up front (small DMAs unblock
            # dense PE work immediately); the rest are interleaved into the
            # norm passes below so norms matmuls fill the build's single-bank
            # WAR stalls and build matmuls fill the norm DMA waits.
            def emit_build_group(kg):
                fv2 = work.tile([128, DP, 512], BF16, tag="fv", bufs=2,
                                name="fv2")
                for c in range(DP):
                    nc.sync.dma_start(fv2[:, c, :], fvT[ts(c, 128), ts(kg, 512)])
                for k4 in range(4):
                    ki = kg * 4 + k4
                    pw = pso.tile([128, M], F32, tag="po", name="pw")
                    for c in range(DP):
                        nc.tensor.matmul(pw[:], fv2[:, c, ts(k4, 128)],
                                         wexp_sb[:, c, :],
                                         start=(c == 0), stop=(c == DP - 1))
                    nc.vector.tensor_add(wt[:, ki, :], pw[:], w0b[:])

            build_groups = iter(range(KB // 4))
            for _ in range(min(3, KB // 4)):
                emit_build_group(next(build_groups))

            def step_build(_ci):
                kg = next(build_groups, None)
                if kg is not None:
                    emit_build_group(kg)

            nc.sync.dma_start(wm_sb[:], wm.rearrange("(c p) m -> p c m", p=128))

            # ---- phase B: fused norms + normalized copies ------------------
            # One pass over the (bf16) data per 512-column chunk: sumsq via
            # square + matmul-with-ones, rsqrt, partition-broadcast, scale.
            # Query data lands directly in the resident rawq (it IS the raw
            # bf16 cast the merge needs); keys use transient staging.
            def norm_scale(src, width, n_row, dst_cn, raw_dst, chunk_done=None):
                for ci, (off, w) in enumerate(_chunks(width, 512)):
                    pn = psg.tile([1, 512], F32, tag="ps", name="pn")
                    sts = []
                    for pt in range(DP):
                        if raw_dst is not None and off + w <= nq:
                            st = raw_dst[:, pt, ds(off, w)]
                        else:
                            st = trans.tile([128, 512], BF16, tag="stf",
                                            bufs=12, name="st")[:, :w]
                        nc.sync.dma_start(st, src[ts(pt, 128), ds(off, w)])
                        sq = work.tile([128, 512], BF16, tag="sqr", bufs=4,
                                       name="sq")
                        nc.vector.tensor_mul(sq[:, :w], st, st)
                        nc.tensor.matmul(pn[:, :w], ones_col[:], sq[:, :w],
                                         start=(pt == 0), stop=(pt == DP - 1))
                        sts.append(st)
                    srow = work.tile([1, 512], F32, tag="srow", name="srow")
                    nc.scalar.sqrt(srow[:, :w], pn[:, :w])
                    nc.vector.reciprocal(n_row[:, ds(off, w)], srow[:, :w])
                    nb = trans.tile([128, 512], F32, tag="nbf", bufs=2, name="nb")
                    nc.gpsimd.partition_broadcast(nb[:, :w], n_row[:, ds(off, w)])
                    for pt in range(DP):
                        nc.vector.tensor_mul(dst_cn[:, pt, ds(off, w)],
                                             sts[pt], nb[:, :w])
                    if chunk_done is not None:
                        chunk_done(ci)

            # ---- phase C: fused sim / counter / merge ----------------------
            # merge(ch-1) is emitted between k-loop(ch) and softplus(ch): the
            # PE chews merge matmuls (whose cts are long ready) while ACT runs
            # softplus(ch); softplus(ch-1) itself overlapped k-loop(ch).
            # Chunk 0's k-iterations are interleaved into the keys norm pass
            # (4 per 512-key chunk) so the PE has gram/z work while the key
            # stream is still loading. The S-relu runs on the DVE so softplus
            # (ACT) never delays the next chunk's relu→z chain.
            def gram_part(ch, ki):
                ps = psg.tile([128, qch], F32, tag="ps", name="ps")
                for dp in range(DP):
                    nc.tensor.matmul(ps[:], cnk[:, dp, ts(ki, 128)],
                                     cnk[:, dp, ds(ch * qch, qch)],
                                     start=(dp == 0), stop=(dp == DP - 1))
                sb = work.tile([128, qch], BF16, tag="sb", bufs=4, name="sb")
                nc.vector.tensor_scalar(sb[:], ps[:], 0.0, None, ALU.max)
                return sb

            def z_part(ki, sb, pz):
                for mj in range(MJ):
                    nc.tensor.matmul(pz[mj][:], wt[:, ki, ts(mj, 128)], sb[:],
                                     start=(ki == 0), stop=(ki == KB - 1))

            class KPipe:
                """Emit z(ki-1) after gram(ki): the PE stream never waits on
                the relu of the tile it is about to consume."""
                def __init__(self, ch, pz):
                    self.ch, self.pz, self.pending = ch, pz, None
                def step(self, ki):
                    sb = gram_part(self.ch, ki)
                    if self.pending is not None:
                        z_part(self.pending[0], self.pending[1], self.pz)
                    self.pending = (ki, sb)
                def flush(self):
                    if self.pending is not None:
                        z_part(self.pending[0], self.pending[1], self.pz)
                        self.pending = None

            def emit_merge(ch, cts):
                for qs in range(qch // 128):
                    po = pso.tile([128, M], F32, tag="po", name="po")
                    for dp in range(DP):
                        nc.tensor.matmul(po[:],
                                         rawq[:, dp, ds(ch * qch + qs * 128, 128)],
                                         wm_sb[:, dp, :],
                                         start=(dp == 0), stop=False)
                    for mj in range(MJ):
                        nc.tensor.matmul(po[:], cts[mj][:, ts(qs, 128)],
                                         wm_sb[:, DP + mj, :],
                                         start=False, stop=(mj == MJ - 1))
                    ob = work.tile([128, M], F32, tag="ob", bufs=2, name="ob")
                    nc.vector.tensor_copy(ob[:], po[:])
                    nc.sync.dma_start(out[ds(ch * qch + qs * 128, 128), :], ob[:])

            def emit_softplus(pz):
                # counter.T = softplus(z + b) = relu(zb) + ln(1 + exp(-|zb|)).
                # Returns (t1, t4) pairs; the final DVE adds are deferred to
                # emit_ct (just before the consuming merge) so the DVE queue
                # at the next chunk's start only holds the pz-freeing t1 ops.
                parts = []
                for mj in range(MJ):
                    bmj = bexp_sb[:, mj:mj + 1]
                    t1 = work.tile([128, qch], F32, tag="t1", bufs=4, name="t1")
                    nc.vector.tensor_scalar(t1[:], pz[mj][:], bmj, 0.0,
                                            ALU.add, ALU.max)
                    t2 = work.tile([128, qch], F32, tag="t2", bufs=2, name="t2")
                    nc.scalar.activation(t2[:], pz[mj][:], AF.Abs, bias=bmj)
                    t3 = work.tile([128, qch], F32, tag="t3", bufs=2, name="t3")
                    nc.scalar.activation(t3[:], t2[:], AF.Exp, scale=-1.0)
                    t4 = work.tile([128, qch], F32, tag="t4", bufs=4, name="t4")
                    nc.scalar.activation(t4[:], t3[:], AF.Ln, bias=1.0)
                    parts.append((t1, t4))
                return parts

            def emit_ct(parts):
                cts = []
                for t1, t4 in parts:
                    ct = work.tile([128, qch], BF16, tag="ct", bufs=4, name="ct")
                    nc.vector.tensor_add(ct[:], t1[:], t4[:])
                    cts.append(ct)
                return cts

            def alloc_pz():
                return [psz.tile([128, qch], F32, tag="pz", name=f"pz{mj}")
                        for mj in range(MJ)]


            # chunk 0: k-work interleaved with the keys norm pass, lagging it
            # by 2 key-chunks so the per-chunk DMA→sumsq→rsqrt→scale latency
            # is hidden behind the PE work of the previous chunks.
            pz0 = alloc_pz()
            LAG = 1

            pipe0 = KPipe(0, pz0)

            def keys_chunk_done(ci):
                step_build(ci)
                cj = ci - LAG
                if cj >= 0:
                    for ki in range(cj * 4, min((cj + 1) * 4, KB)):
                        pipe0.step(ki)

            norm_scale(dTk, nkeys, n_row_k, cnk, rawq,
                       chunk_done=keys_chunk_done)
            for kg in build_groups:
                emit_build_group(kg)
            for cj in range(max(0, nkeys // 512 - LAG), nkeys // 512):
                for ki in range(cj * 4, min((cj + 1) * 4, KB)):
                    pipe0.step(ki)
            pipe0.flush()
            prev = emit_softplus(pz0)

            for ch in range(1, NCH):
                pz = alloc_pz()
                pipe = KPipe(ch, pz)
                for ki in range(KB):
                    pipe.step(ki)
                pipe.flush()
                emit_merge(ch - 1, emit_ct(prev))
                prev = emit_softplus(pz)
            emit_merge(NCH - 1, emit_ct(prev))

    nc.compile()
    return nc


def make_in_maps(data, W_exp, b_exp, W_merge, num_cores=NCORES):
    """Host prep: transpose/slice/cast inputs into per-core input maps."""
    data = np.asarray(data, dtype=np.float32)
    W_exp = np.asarray(W_exp, dtype=np.float32)
    b_exp = np.asarray(b_exp, dtype=np.float32)
    W_merge = np.asarray(W_merge, dtype=np.float32)

    dataT = np.ascontiguousarray(data.transpose(0, 2, 1)).astype(BF)  # [B,D,N]
    fvT_bf = np.ascontiguousarray(_posenc(N, D).T).astype(BF)
    wexp1_bf = W_exp[1:].astype(BF)
    w0 = np.ascontiguousarray(W_exp[0:1])
    wm_bf = W_merge.astype(BF)
    bexp_r = np.ascontiguousarray(b_exp.reshape(M // 128, 128))

    fvT_rot = np.ascontiguousarray(np.roll(fvT_bf, -NQ, axis=1))
    in_maps = []
    for c in range(num_cores):
        b, h = c // 2, c % 2
        # rotate key columns so this core's query rows are always keys
        # [0:NQ]; fvT is rotated identically (the k-sum is permutation-
        # invariant and Wt is built from the same rotated fvT).
        in_maps.append({
            "dTk": dataT[b] if h == 0 else np.ascontiguousarray(
                np.roll(dataT[b], -NQ, axis=1)),
            "fvT": fvT_bf if h == 0 else fvT_rot,
            "wexp1": wexp1_bf,
            "w0": w0,
            "wm": wm_bf,
            "bexp": bexp_r,
        })
    return in_maps


_NC_CACHE = {}


def get_nc():
    if "full" not in _NC_CACHE:
        _NC_CACHE["full"] = build_nc()
    return _NC_CACHE["full"]


def kernel(data, W_exp, b_exp, W_merge):
    nc = get_nc()
    in_maps = make_in_maps(data, W_exp, b_exp, W_merge)
    res = run_bass_kernel_spmd(nc, in_maps, core_ids=list(range(NCORES)))
    out = np.empty((B, N, M), dtype=np.float32)
    for c in range(NCORES):
        b, h = c // 2, c % 2
        out[b, h * NQ:(h + 1) * NQ] = res.results[c]["out"]
    return out

